# revision 23
# baseline (speedup 1.0000x reference)
"""Trainium2 Bass kernel for a BFP-quantized ResNet BasicBlock (inference).

Computes, per image (NCHW, C=128, H=W=56):
    out = relu( bn2( conv3x3( q( relu(bn1( conv3x3(q(x), q(w1)) )) ), q(w2)) ) + x )
where q() is HBFP block-floating-point quantization: blocks of 64 contiguous
values (in flat row-major order) share a power-of-2 scale 2^(floor(log2(max|x|))-7),
mantissas rounded (RNE) to 8 signed bits and clamped to +-127.

Key facts exploited:
  * Quantized values are (int in [-127,127]) * 2^k  -> exactly representable in
    bf16, so convs run on the PE at bf16 speed with zero extra error.
  * floor(log2(m)) for normal floats == exponent-field extraction (bitwise ops).
  * RNE rounding == (t + 1.5*2**23) - 1.5*2**23 in fp32 (one dual-op tensor_scalar).
  * conv3x3 = 9 accumulated matmuls (C_in=128 on partitions) over a zero-padded
    58-pitch image layout, fully contiguous rhs slices of 464 columns (8 rows).
  * bn2's scale inv2 is folded into the quantized w2 at setup (one setup op),
    so conv2's eviction is scale-free and out = relu(t2 + x) needs only one
    image-level VectorE add and one ScalarE relu.
  * Weight lhsT tiles are produced by DMA-XBAR transposes, so the PE queue
    begins directly with conv work, and the first x-image load + quant is
    interleaved with the weight setup chains (short pipeline fill).

Pipeline: conv2 lags conv1 by TWO images on the PE (c1(0), c1(1), c1(2),
c2(0), c1(3), c2(1), ...) so the ~22us serial quant chain (absmax reduce ->
exponent smalls -> rscale-mult -> clip -> RNE round -> bf16 scale -> padded
copy) of image k is covered by ~2.5 convs of PE work. Engine split per
~28us image cycle: VectorE reduce/exponent-smalls/round/scale + residual add;
GpSimd rscale-mult + clip (Pool cannot run bitwise ops and is ~10x slow on
dtype-converting dual ops and tiny ops, so only the two big f32 ops live
there); ScalarE PSUM evictions + padded copy + relu + scale-cast.

Sharding: data-parallel over batch N=64 -> 8 images per NeuronCore, weights and
BN constants replicated. All 8 cores run the same NEFF (SPMD).
"""

import os

os.environ.setdefault("MYCRO_LOCAL_CACHE", "1")

from contextlib import ExitStack
from functools import lru_cache

import numpy as np

import concourse.bass as bass
import concourse.tile as tile
from concourse import bacc, mybir
from concourse.bass_utils import run_bass_kernel_spmd

P = 128
H = W = 56
HWF = H * W            # 3136 flat pixels per channel
NBX = HWF // 64        # 49 BFP blocks per channel image
WLEN = 128 * 9         # 1152 flat weight row per output channel
NBW = WLEN // 64       # 18 BFP blocks per weight row
PITCH = W + 2          # 58 padded row pitch
PADLEN = PITCH * PITCH + 2  # 3366: [1 pre-pad][58x58 padded image][1 post-pad]
NCH = 7                # 8-row output chunks per image
CH = 8 * W             # 448 useful outputs per chunk
CHF = 8 * PITCH        # 464 matmul free dim per chunk
CROUND = 12582912.0    # 1.5 * 2**23  (RNE magic constant)
CLIPV = 127.4
EXPMASK = 0x7F800000
BIAS7 = 7 << 23
C254 = 254 << 23
EGUARD = 50 << 23      # exponent field of 1e-23 (the reference's zero-guard)
BN_EPS = 1e-5

F32 = mybir.dt.float32
BF16 = mybir.dt.bfloat16
I32 = mybir.dt.int32
ALU = mybir.AluOpType
ACTF = mybir.ActivationFunctionType
AX = mybir.AxisListType

N_CORES = 8
NIMG = 8  # images per core


def _emit_quant(nc, small, tmp, src_ap, dst_ap, nb):
    """BFP-quantize src_ap (f32 [P, nb*64]) into dst_ap (bf16 [P, nb*64]).

    VectorE: absmax reduce, exponent smalls, RNE round, bf16 scale-mult.
    GpSimd: the two big f32 ops (rscale-mult, clip). ScalarE: the bf16
    scale cast.
    """
    src3 = src_ap.rearrange("p (b e) -> p b e", e=64)
    dst3 = dst_ap.rearrange("p (b e) -> p b e", e=64)

    bm = small.tile([P, nb], F32, tag=f"bm{nb}")
    sb = small.tile([P, nb], I32, tag=f"sb{nb}")
    rb = small.tile([P, nb], I32, tag=f"rb{nb}")
    sc_bf = small.tile([P, nb], BF16, tag=f"scbf{nb}")
    t = tmp.tile([P, nb * 64], F32, tag=f"qt{nb}")
    t3 = t[:].rearrange("p (b e) -> p b e", e=64)

    nc.vector.tensor_reduce(
        out=bm[:], in_=src3, axis=AX.X, op=ALU.max, apply_absolute_value=True,
    )
    # scale bits = max(exponent field, expfield(1e-23)) - (7 << 23)
    # (the max reproduces the reference's +1e-23 zero-guard; dual-op
    # tensor_scalars cannot mix bitwise and arith ops, so the AND is alone)
    nc.vector.tensor_scalar(sb[:], bm[:].bitcast(I32), EXPMASK, None,
                            ALU.bitwise_and)
    nc.vector.tensor_scalar(sb[:], sb[:], EGUARD, BIAS7, ALU.max, ALU.subtract)
    # rscale bits = (254 << 23) - scale_bits  -> rscale = 2^(7-e) = 1/scale
    nc.vector.tensor_scalar(rb[:], sb[:], C254, -1, ALU.subtract, ALU.mult)
    nc.scalar.copy(sc_bf[:], sb[:].bitcast(F32))
    rsc = rb[:].bitcast(F32)[:, :, None].to_broadcast((P, nb, 64))
    nc.gpsimd.tensor_tensor(t3, src3, rsc, ALU.mult)
    nc.gpsimd.tensor_scalar(t3, t3, CLIPV, -CLIPV, ALU.min, ALU.max)
    # RNE round; result is a small integer -> exact in bf16
    nc.vector.tensor_scalar(dst3, t3, CROUND, CROUND, ALU.add, ALU.subtract)
    scb = sc_bf[:][:, :, None].to_broadcast((P, nb, 64))
    nc.vector.tensor_tensor(dst3, dst3, scb, ALU.mult)


def _padview(pad_tile):
    """[P, 58, 58] view of the padded image (pitch 58, 1-element pre-pad)."""
    return pad_tile[:, 1 : 1 + PITCH * PITCH].rearrange(
        "p (r w) -> p r w", w=PITCH)


def _interior(pad_tile):
    """[P, 56, 56] strided view of the padded tile's interior."""
    return _padview(pad_tile)[:, 1 : 1 + H, 1 : 1 + W]


def _emit_conv(nc, psum_pool, wk, pad_tile, evict):
    """3x3 conv via 9 accumulated matmuls per 8-row chunk over contiguous
    464-column rhs slices."""
    for c in range(NCH):
        ps = psum_pool.tile([P, CHF], F32, tag="pschunk")
        for k in range(9):
            kh, kw = divmod(k, 3)
            s = (8 * c + kh) * PITCH + kw
            nc.tensor.matmul(
                ps[:], wk[k][:], pad_tile[:, s : s + CHF],
                start=(k == 0), stop=(k == 8),
            )
        evict(c, ps)


def _psv(ps):
    """[P, 8, 56] useful-interior view of a [P, 464] PSUM chunk."""
    return ps[:].rearrange("p (r w) -> p r w", w=PITCH)[:, :, 1 : 1 + W]


def build_nc(nimg=NIMG):
    nc = bacc.Bacc("TRN2", target_bir_lowering=False, debug=False,
                   enable_asserts=False)

    x_d = nc.dram_tensor("x", [nimg, P, H, W], F32, kind="ExternalInput").ap()
    w1_d = nc.dram_tensor("w1", [P, P, 3, 3], F32, kind="ExternalInput").ap()
    w2_d = nc.dram_tensor("w2", [P, P, 3, 3], F32, kind="ExternalInput").ap()
    bn_d = {
        name: nc.dram_tensor(name, [P], F32, kind="ExternalInput").ap()
        for name in ("gamma1", "beta1", "mean1", "var1",
                     "gamma2", "beta2", "mean2", "var2")
    }
    out_d = nc.dram_tensor("out", [nimg, P, H, W], F32, kind="ExternalOutput").ap()

    with tile.TileContext(nc) as tc, ExitStack() as ctx:
        const = ctx.enter_context(tc.tile_pool(name="const", bufs=1))
        small = ctx.enter_context(tc.tile_pool(name="small", bufs=4))
        tmp = ctx.enter_context(tc.tile_pool(name="tmp", bufs=2))
        pads = ctx.enter_context(tc.tile_pool(name="pads", bufs=1))
        wsetup = ctx.enter_context(tc.tile_pool(name="wsetup", bufs=1))
        xraw_p = ctx.enter_context(tc.tile_pool(name="xraw", bufs=4))
        u_p = ctx.enter_context(tc.tile_pool(name="u", bufs=2))
        mid_p = ctx.enter_context(tc.tile_pool(name="mid", bufs=2))
        t2_p = ctx.enter_context(tc.tile_pool(name="t2", bufs=2))
        u2_p = ctx.enter_context(tc.tile_pool(name="u2", bufs=2))
        psum1_p = ctx.enter_context(tc.tile_pool(name="psum1", bufs=3, space="PSUM"))
        psum2_p = ctx.enter_context(tc.tile_pool(name="psum2", bufs=3, space="PSUM"))
        psumt_p = ctx.enter_context(tc.tile_pool(name="psumt", bufs=2, space="PSUM"))

        def setup_bn():
            """BN constants; DMAs go on the scalar queue so the sync queue
            serves the latency-critical first x-image load immediately."""
            ident = const.tile([P, P], BF16, tag="ident")
            from concourse.masks import make_identity
            make_identity(nc, ident[:])
            eps_b = small.tile([P, 1], F32, tag="eps_b")
            nc.vector.memset(eps_b[:], BN_EPS)
            bnc = {}
            for name in ("gamma1", "beta1", "mean1", "var1",
                         "gamma2", "beta2", "mean2", "var2"):
                t = small.tile([P, 1], F32, tag=f"bn_{name}")
                nc.scalar.dma_start(t[:], bn_d[name][:, None])
                bnc[name] = t
            invb = []
            for i in ("1", "2"):
                s = small.tile([P, 1], F32, tag=f"sd{i}")
                nc.scalar.activation(s[:], bnc[f"var{i}"][:], ACTF.Sqrt, bias=eps_b[:])
                r = small.tile([P, 1], F32, tag=f"rs{i}")
                nc.vector.reciprocal(r[:], s[:])
                inv = const.tile([P, 1], F32, tag=f"inv{i}")
                nc.vector.tensor_tensor(inv[:], bnc[f"gamma{i}"][:], r[:], ALU.mult)
                mi = small.tile([P, 1], F32, tag=f"mi{i}")
                nc.vector.tensor_tensor(mi[:], bnc[f"mean{i}"][:], inv[:], ALU.mult)
                b = const.tile([P, 1], F32, tag=f"b{i}")
                nc.vector.tensor_tensor(b[:], bnc[f"beta{i}"][:], mi[:], ALU.subtract)
                invb.append((inv, b))
            return ident, invb

        def setup_weights(wi, w_d):
            """Quantize w{1,2} and build the 9 lhsT tiles via PE transposes."""
            wraw = wsetup.tile([P, WLEN], F32, tag="wraw")
            nc.scalar.dma_start(wraw[:], w_d.rearrange("o i kh kw -> o (i kh kw)"))
            wq = wsetup.tile([P, WLEN], BF16, tag=f"wq{wi}")
            _emit_quant(nc, small, wsetup, wraw[:], wq[:], NBW)
            if wi == 1:
                # fold bn2's scale into w2 so conv2's PSUM = inv2*conv2
                nc.vector.tensor_scalar(wq[:], wq[:], inv2[:], None, ALU.mult)
            # per-offset lhsT tiles: w[k][i, o] = wq[o, i*9+k]; PE transposes
            # (DMA-XBAR transpose rejects the stride-9 source view) through
            # psum2_p, which is idle until conv2(0)
            wq_v = wq[:].rearrange("p (i k) -> p k i", k=9)
            wk = []
            for k in range(9):
                pt = psumt_p.tile([P, P], BF16, tag="tps")
                nc.tensor.transpose(pt[:], wq_v[:, k, :], ident[:])
                wt = const.tile([P, P], BF16, tag=f"w{wi}k{k}")
                nc.scalar.copy(wt[:], pt[:])
                wk.append(wt)
            return wk

        xq_pads = [pads.tile([P, PADLEN], BF16, tag=f"xqp{i}", name=f"xqp{i}")
                   for i in range(2)]
        mq_pads = [pads.tile([P, PADLEN], BF16, tag=f"mqp{i}", name=f"mqp{i}")
                   for i in range(2)]
        for t in (*xq_pads, *mq_pads):
            # border-only zeroing (interior is overwritten every image):
            # pre-pad + top row, bottom row + post-pad, left col, right col
            pv = _padview(t)
            nc.gpsimd.memset(t[:, 0 : 1 + PITCH], 0.0)
            nc.gpsimd.memset(t[:, 1 + PITCH * (PITCH - 1) : PADLEN], 0.0)
            nc.gpsimd.memset(pv[:, 1 : PITCH - 1, 0:1], 0.0)
            nc.gpsimd.memset(pv[:, 1 : PITCH - 1, PITCH - 1 : PITCH], 0.0)

        xraws = [None] * nimg
        mids = [None] * nimg
        t2s = [None] * nimg

        def load_quant1(n):
            xr = xraw_p.tile([P, HWF], F32, tag="xraw", name=f"xraw{n}")
            xraws[n] = xr
            nc.sync.dma_start(xr[:], x_d[n].rearrange("c h w -> c (h w)"))
            u = u_p.tile([P, HWF], BF16, tag="u", name=f"u{n}")
            _emit_quant(nc, small, tmp, xr[:], u[:], NBX)
            nc.sync.dma_start(_interior(xq_pads[n % 2]),
                              u[:].rearrange("p (h w) -> p h w", w=W))

        def conv1(n):
            mid = mid_p.tile([P, HWF], F32, tag="mid", name=f"mid{n}")
            mids[n] = mid

            def evict1(c, ps):
                ov = mid[:, c * CH : (c + 1) * CH].rearrange(
                    "p (r w) -> p r w", w=W)
                nc.scalar.activation(ov, _psv(ps), ACTF.Relu,
                                     bias=b1[:], scale=inv1[:])

            _emit_conv(nc, psum1_p, w1k, xq_pads[n % 2], evict1)

        def quant2(n):
            u2 = u2_p.tile([P, HWF], BF16, tag="u2", name=f"u2_{n}")
            _emit_quant(nc, small, tmp, mids[n][:], u2[:], NBX)
            nc.scalar.copy(_interior(mq_pads[n % 2]),
                           u2[:].rearrange("p (h w) -> p h w", w=W))

        def conv2(n):
            t2 = t2_p.tile([P, HWF], F32, tag="t2", name=f"t2_{n}")
            t2s[n] = t2

            def evict2(c, ps):
                ov = t2[:, c * CH : (c + 1) * CH].rearrange(
                    "p (r w) -> p r w", w=W)
                nc.scalar.activation(ov, _psv(ps), ACTF.Identity, bias=b2[:])

            _emit_conv(nc, psum2_p, w2k, mq_pads[n % 2], evict2)

        def final(n):
            # out = relu(t2 + x); the image-level batch add keeps VectorE's
            # in-order queue free of per-chunk PE-gated deadlines
            t2 = t2s[n]
            nc.vector.tensor_tensor(t2[:], t2[:], xraws[n][:], ALU.add)
            nc.scalar.activation(t2[:], t2[:], ACTF.Relu)
            nc.scalar.dma_start(out_d[n].rearrange("c h w -> c (h w)"), t2[:])

        # Emission interleaves the weight setup with the first image loads so
        # the pipeline fill is short; PE order is c1(0), c1(1), c1(2), c2(0),
        # c1(3), c2(1), ... (conv2 lags by two images, covering the quant2
        # latency with ~2.5 convs of PE work).
        load_quant1(0)
        ident, invb = setup_bn()
        (inv1, b1), (inv2, b2) = invb
        w1k = setup_weights(0, w1_d)
        load_quant1(1)
        conv1(0)
        w2k = setup_weights(1, w2_d)
        load_quant1(2)
        quant2(0)
        conv1(1)
        load_quant1(3)
        quant2(1)
        for n in range(2, nimg):
            conv1(n)
            conv2(n - 2)
            quant2(n)
            final(n - 2)
            if n + 2 < nimg:
                load_quant1(n + 2)
        conv2(nimg - 2)
        final(nimg - 2)
        conv2(nimg - 1)
        final(nimg - 1)

    nc.compile()
    return nc


@lru_cache(maxsize=1)
def _get_nc():
    return build_nc(NIMG)


def kernel(x, w1, w2, gamma1, beta1, mean1, var1,
           gamma2, beta2, mean2, var2, _trace=False):
    f = lambda a: np.ascontiguousarray(np.asarray(a, dtype=np.float32))
    x = f(x)
    n_total = x.shape[0]
    assert n_total == N_CORES * NIMG, x.shape
    xs = x.reshape(N_CORES, NIMG, P, H, W)
    rep = {
        "w1": f(w1), "w2": f(w2),
        "gamma1": f(gamma1), "beta1": f(beta1), "mean1": f(mean1), "var1": f(var1),
        "gamma2": f(gamma2), "beta2": f(beta2), "mean2": f(mean2), "var2": f(var2),
    }
    in_maps = [{"x": np.ascontiguousarray(xs[c]), **rep} for c in range(N_CORES)]
    nc = _get_nc()
    res = run_bass_kernel_spmd(nc, in_maps, core_ids=list(range(N_CORES)),
                               trace=_trace)
    out = np.concatenate([res.results[c]["out"] for c in range(N_CORES)], axis=0)
    if _trace:
        kernel.last_result = res
    return out.reshape(n_total, P, H, W)


# revision 26
# speedup vs baseline: 1.0616x; 1.0616x over previous
"""Trainium2 Bass kernel for a BFP-quantized ResNet BasicBlock (inference).

Computes, per image (NCHW, C=128, H=W=56):
    out = relu( bn2( conv3x3( q( relu(bn1( conv3x3(q(x), q(w1)) )) ), q(w2)) ) + x )
where q() is HBFP block-floating-point quantization: blocks of 64 contiguous
values (in flat row-major order) share a power-of-2 scale 2^(floor(log2(max|x|))-7),
mantissas rounded (RNE) to 8 signed bits and clamped to +-127.

Key facts exploited:
  * Quantized values are (int in [-127,127]) * 2^k  -> exactly representable in
    bf16, so convs run on the PE at bf16 speed with zero extra error.
  * floor(log2(m)) for normal floats == exponent-field extraction (bitwise ops).
  * RNE rounding == (t + 1.5*2**23) - 1.5*2**23 in fp32 (one dual-op tensor_scalar).
  * conv3x3 = 9 accumulated matmuls (C_in=128 on partitions) over a zero-padded
    58-pitch image layout, fully contiguous rhs slices of 464 columns (8 rows).
  * bn2's scale inv2 is folded into the quantized w2 at setup (one setup op),
    so conv2's eviction is scale-free and out = relu(t2 + x) needs only one
    image-level VectorE add and one ScalarE relu.
  * Weight lhsT tiles are produced by DMA-XBAR transposes, so the PE queue
    begins directly with conv work, and the first x-image load + quant is
    interleaved with the weight setup chains (short pipeline fill).

Pipeline: conv2 lags conv1 by TWO images on the PE (c1(0), c1(1), c1(2),
c2(0), c1(3), c2(1), ...) so the ~22us serial quant chain (absmax reduce ->
exponent smalls -> rscale-mult -> clip -> RNE round -> bf16 scale -> padded
copy) of image k is covered by ~2.5 convs of PE work. Engine split per
~28us image cycle: VectorE reduce/exponent-smalls/round/scale + residual add;
GpSimd rscale-mult + clip (Pool cannot run bitwise ops and is ~10x slow on
dtype-converting dual ops and tiny ops, so only the two big f32 ops live
there); ScalarE PSUM evictions + padded copy + relu + scale-cast.

Sharding: data-parallel over batch N=64 -> 8 images per NeuronCore, weights and
BN constants replicated. All 8 cores run the same NEFF (SPMD).
"""

import os

os.environ.setdefault("MYCRO_LOCAL_CACHE", "1")

from contextlib import ExitStack
from functools import lru_cache

import numpy as np

import concourse.bass as bass
import concourse.tile as tile
from concourse import bacc, mybir
from concourse.bass_utils import run_bass_kernel_spmd

P = 128
H = W = 56
HWF = H * W            # 3136 flat pixels per channel
NBX = HWF // 64        # 49 BFP blocks per channel image
WLEN = 128 * 9         # 1152 flat weight row per output channel
NBW = WLEN // 64       # 18 BFP blocks per weight row
PITCH = W + 2          # 58 padded row pitch
PADLEN = PITCH * PITCH + 2  # 3366: [1 pre-pad][58x58 padded image][1 post-pad]
NCH = 7                # 8-row output chunks per image
CH = 8 * W             # 448 useful outputs per chunk
CHF = 8 * PITCH        # 464 matmul free dim per chunk
CROUND = 12582912.0    # 1.5 * 2**23  (RNE magic constant)
CLIPV = 127.4
EXPMASK = 0x7F800000
BIAS7 = 7 << 23
C254 = 254 << 23
EGUARD = 50 << 23      # exponent field of 1e-23 (the reference's zero-guard)
BN_EPS = 1e-5

F32 = mybir.dt.float32
BF16 = mybir.dt.bfloat16
I32 = mybir.dt.int32
ALU = mybir.AluOpType
ACTF = mybir.ActivationFunctionType
AX = mybir.AxisListType

N_CORES = 8
NIMG = 8  # images per core


def _emit_quant(nc, small, tmp, src_ap, dst_ap, nb):
    """BFP-quantize src_ap (f32 [P, nb*64]) into dst_ap (bf16 [P, nb*64]).

    VectorE: absmax reduce, exponent smalls, RNE round, bf16 scale-mult.
    GpSimd: the two big f32 ops (rscale-mult, clip). ScalarE: the bf16
    scale cast.
    """
    src3 = src_ap.rearrange("p (b e) -> p b e", e=64)
    dst3 = dst_ap.rearrange("p (b e) -> p b e", e=64)

    bm = small.tile([P, nb], F32, tag=f"bm{nb}")
    sb = small.tile([P, nb], I32, tag=f"sb{nb}")
    rb = small.tile([P, nb], I32, tag=f"rb{nb}")
    sc_bf = small.tile([P, nb], BF16, tag=f"scbf{nb}")
    t = tmp.tile([P, nb * 64], F32, tag=f"qt{nb}")
    t3 = t[:].rearrange("p (b e) -> p b e", e=64)

    nc.vector.tensor_reduce(
        out=bm[:], in_=src3, axis=AX.X, op=ALU.max, apply_absolute_value=True,
    )
    # scale bits = max(exponent field, expfield(1e-23)) - (7 << 23)
    # (the max reproduces the reference's +1e-23 zero-guard; dual-op
    # tensor_scalars cannot mix bitwise and arith ops, so the AND is alone)
    nc.vector.tensor_scalar(sb[:], bm[:].bitcast(I32), EXPMASK, None,
                            ALU.bitwise_and)
    nc.vector.tensor_scalar(sb[:], sb[:], EGUARD, BIAS7, ALU.max, ALU.subtract)
    # rscale bits = (254 << 23) - scale_bits  -> rscale = 2^(7-e) = 1/scale
    nc.vector.tensor_scalar(rb[:], sb[:], C254, -1, ALU.subtract, ALU.mult)
    nc.scalar.copy(sc_bf[:], sb[:].bitcast(F32))
    rsc = rb[:].bitcast(F32)[:, :, None].to_broadcast((P, nb, 64))
    nc.gpsimd.tensor_tensor(t3, src3, rsc, ALU.mult)
    # RNE round; result is a small integer -> exact in bf16; the mantissa
    # clamp runs after the round on the bf16 integers (exactly equivalent
    # to clip-then-round, and half the traffic of an f32 in-place clip)
    nc.vector.tensor_scalar(dst3, t3, CROUND, CROUND, ALU.add, ALU.subtract)
    nc.vector.tensor_scalar(dst3, dst3, 127.0, -127.0, ALU.min, ALU.max)
    scb = sc_bf[:][:, :, None].to_broadcast((P, nb, 64))
    nc.vector.tensor_tensor(dst3, dst3, scb, ALU.mult)


def _padview(pad_tile):
    """[P, 58, 58] view of the padded image (pitch 58, 1-element pre-pad)."""
    return pad_tile[:, 1 : 1 + PITCH * PITCH].rearrange(
        "p (r w) -> p r w", w=PITCH)


def _interior(pad_tile):
    """[P, 56, 56] strided view of the padded tile's interior."""
    return _padview(pad_tile)[:, 1 : 1 + H, 1 : 1 + W]


def _emit_conv(nc, psum_pool, wk, pad_tile, evict):
    """3x3 conv via 9 accumulated matmuls per 8-row chunk over contiguous
    464-column rhs slices."""
    for c in range(NCH):
        ps = psum_pool.tile([P, CHF], F32, tag="pschunk")
        for k in range(9):
            kh, kw = divmod(k, 3)
            s = (8 * c + kh) * PITCH + kw
            nc.tensor.matmul(
                ps[:], wk[k][:], pad_tile[:, s : s + CHF],
                start=(k == 0), stop=(k == 8),
            )
        evict(c, ps)


def _psv(ps):
    """[P, 8, 56] useful-interior view of a [P, 464] PSUM chunk."""
    return ps[:].rearrange("p (r w) -> p r w", w=PITCH)[:, :, 1 : 1 + W]


def build_nc(nimg=NIMG):
    nc = bacc.Bacc("TRN2", target_bir_lowering=False, debug=False,
                   enable_asserts=False)

    x_d = nc.dram_tensor("x", [nimg, P, H, W], F32, kind="ExternalInput").ap()
    w1_d = nc.dram_tensor("w1", [P, P, 3, 3], F32, kind="ExternalInput").ap()
    w2_d = nc.dram_tensor("w2", [P, P, 3, 3], F32, kind="ExternalInput").ap()
    bn_d = {
        name: nc.dram_tensor(name, [P], F32, kind="ExternalInput").ap()
        for name in ("gamma1", "beta1", "mean1", "var1",
                     "gamma2", "beta2", "mean2", "var2")
    }
    out_d = nc.dram_tensor("out", [nimg, P, H, W], F32, kind="ExternalOutput").ap()

    with tile.TileContext(nc) as tc, ExitStack() as ctx:
        const = ctx.enter_context(tc.tile_pool(name="const", bufs=1))
        small = ctx.enter_context(tc.tile_pool(name="small", bufs=4))
        tmp = ctx.enter_context(tc.tile_pool(name="tmp", bufs=2))
        pads = ctx.enter_context(tc.tile_pool(name="pads", bufs=1))
        wsetup = ctx.enter_context(tc.tile_pool(name="wsetup", bufs=1))
        xraw_p = ctx.enter_context(tc.tile_pool(name="xraw", bufs=4))
        u_p = ctx.enter_context(tc.tile_pool(name="u", bufs=2))
        mid_p = ctx.enter_context(tc.tile_pool(name="mid", bufs=2))
        t2_p = ctx.enter_context(tc.tile_pool(name="t2", bufs=2))
        u2_p = ctx.enter_context(tc.tile_pool(name="u2", bufs=2))
        psum1_p = ctx.enter_context(tc.tile_pool(name="psum1", bufs=3, space="PSUM"))
        psum2_p = ctx.enter_context(tc.tile_pool(name="psum2", bufs=3, space="PSUM"))
        psumt_p = ctx.enter_context(tc.tile_pool(name="psumt", bufs=2, space="PSUM"))

        def setup_bn():
            """BN constants; DMAs go on the scalar queue so the sync queue
            serves the latency-critical first x-image load immediately."""
            ident = const.tile([P, P], BF16, tag="ident")
            from concourse.masks import make_identity
            make_identity(nc, ident[:])
            eps_b = small.tile([P, 1], F32, tag="eps_b")
            nc.vector.memset(eps_b[:], BN_EPS)
            bnc = {}
            for name in ("gamma1", "beta1", "mean1", "var1",
                         "gamma2", "beta2", "mean2", "var2"):
                t = small.tile([P, 1], F32, tag=f"bn_{name}")
                nc.scalar.dma_start(t[:], bn_d[name][:, None])
                bnc[name] = t
            invb = []
            for i in ("1", "2"):
                s = small.tile([P, 1], F32, tag=f"sd{i}")
                nc.scalar.activation(s[:], bnc[f"var{i}"][:], ACTF.Sqrt, bias=eps_b[:])
                r = small.tile([P, 1], F32, tag=f"rs{i}")
                nc.vector.reciprocal(r[:], s[:])
                inv = const.tile([P, 1], F32, tag=f"inv{i}")
                nc.vector.tensor_tensor(inv[:], bnc[f"gamma{i}"][:], r[:], ALU.mult)
                mi = small.tile([P, 1], F32, tag=f"mi{i}")
                nc.vector.tensor_tensor(mi[:], bnc[f"mean{i}"][:], inv[:], ALU.mult)
                b = const.tile([P, 1], F32, tag=f"b{i}")
                nc.vector.tensor_tensor(b[:], bnc[f"beta{i}"][:], mi[:], ALU.subtract)
                invb.append((inv, b))
            return ident, invb

        def setup_weights(wi, w_d):
            """Quantize w{1,2} and build the 9 lhsT tiles via PE transposes."""
            wraw = wsetup.tile([P, WLEN], F32, tag="wraw")
            nc.scalar.dma_start(wraw[:], w_d.rearrange("o i kh kw -> o (i kh kw)"))
            wq = wsetup.tile([P, WLEN], BF16, tag=f"wq{wi}")
            _emit_quant(nc, small, wsetup, wraw[:], wq[:], NBW)
            if wi == 1:
                # fold bn2's scale into w2 so conv2's PSUM = inv2*conv2
                nc.vector.tensor_scalar(wq[:], wq[:], inv2[:], None, ALU.mult)
            # per-offset lhsT tiles: w[k][i, o] = wq[o, i*9+k]; PE transposes
            # (DMA-XBAR transpose rejects the stride-9 source view) through
            # psum2_p, which is idle until conv2(0)
            wq_v = wq[:].rearrange("p (i k) -> p k i", k=9)
            wk = []
            for k in range(9):
                pt = psumt_p.tile([P, P], BF16, tag="tps")
                nc.tensor.transpose(pt[:], wq_v[:, k, :], ident[:])
                wt = const.tile([P, P], BF16, tag=f"w{wi}k{k}")
                nc.scalar.copy(wt[:], pt[:])
                wk.append(wt)
            return wk

        xq_pads = [pads.tile([P, PADLEN], BF16, tag=f"xqp{i}", name=f"xqp{i}")
                   for i in range(2)]
        mq_pads = [pads.tile([P, PADLEN], BF16, tag=f"mqp{i}", name=f"mqp{i}")
                   for i in range(2)]
        for t in (*xq_pads, *mq_pads):
            # border-only zeroing (interior is overwritten every image):
            # pre-pad + top row, bottom row + post-pad, left col, right col
            pv = _padview(t)
            nc.gpsimd.memset(t[:, 0 : 1 + PITCH], 0.0)
            nc.gpsimd.memset(t[:, 1 + PITCH * (PITCH - 1) : PADLEN], 0.0)
            nc.gpsimd.memset(pv[:, 1 : PITCH - 1, 0:1], 0.0)
            nc.gpsimd.memset(pv[:, 1 : PITCH - 1, PITCH - 1 : PITCH], 0.0)

        xraws = [None] * nimg
        mids = [None] * nimg
        t2s = [None] * nimg

        def load_quant1(n):
            xr = xraw_p.tile([P, HWF], F32, tag="xraw", name=f"xraw{n}")
            xraws[n] = xr
            nc.sync.dma_start(xr[:], x_d[n].rearrange("c h w -> c (h w)"))
            u = u_p.tile([P, HWF], BF16, tag="u", name=f"u{n}")
            _emit_quant(nc, small, tmp, xr[:], u[:], NBX)
            nc.sync.dma_start(_interior(xq_pads[n % 2]),
                              u[:].rearrange("p (h w) -> p h w", w=W))

        def conv1(n):
            mid = mid_p.tile([P, HWF], F32, tag="mid", name=f"mid{n}")
            mids[n] = mid

            def evict1(c, ps):
                ov = mid[:, c * CH : (c + 1) * CH].rearrange(
                    "p (r w) -> p r w", w=W)
                nc.scalar.activation(ov, _psv(ps), ACTF.Relu,
                                     bias=b1[:], scale=inv1[:])

            _emit_conv(nc, psum1_p, w1k, xq_pads[n % 2], evict1)

        def quant2(n):
            u2 = u2_p.tile([P, HWF], BF16, tag="u2", name=f"u2_{n}")
            _emit_quant(nc, small, tmp, mids[n][:], u2[:], NBX)
            nc.scalar.copy(_interior(mq_pads[n % 2]),
                           u2[:].rearrange("p (h w) -> p h w", w=W))

        def conv2(n):
            t2 = t2_p.tile([P, HWF], F32, tag="t2", name=f"t2_{n}")
            t2s[n] = t2

            def evict2(c, ps):
                ov = t2[:, c * CH : (c + 1) * CH].rearrange(
                    "p (r w) -> p r w", w=W)
                nc.scalar.activation(ov, _psv(ps), ACTF.Identity, bias=b2[:])

            _emit_conv(nc, psum2_p, w2k, mq_pads[n % 2], evict2)

        def final(n):
            # out = relu(t2 + x); the image-level batch add keeps VectorE's
            # in-order queue free of per-chunk PE-gated deadlines
            t2 = t2s[n]
            nc.vector.tensor_tensor(t2[:], t2[:], xraws[n][:], ALU.add)
            nc.scalar.activation(t2[:], t2[:], ACTF.Relu)
            nc.scalar.dma_start(out_d[n].rearrange("c h w -> c (h w)"), t2[:])

        # Emission interleaves the weight setup with the first image loads so
        # the pipeline fill is short; PE order is c1(0), c1(1), c1(2), c2(0),
        # c1(3), c2(1), ... (conv2 lags by two images, covering the quant2
        # latency with ~2.5 convs of PE work).
        with tc.high_priority():
            load_quant1(0)
        ident, invb = setup_bn()
        (inv1, b1), (inv2, b2) = invb
        w1k = setup_weights(0, w1_d)
        load_quant1(1)
        conv1(0)
        w2k = setup_weights(1, w2_d)
        load_quant1(2)
        quant2(0)
        conv1(1)
        load_quant1(3)
        quant2(1)
        for n in range(2, nimg):
            conv1(n)
            conv2(n - 2)
            quant2(n)
            final(n - 2)
            if n + 2 < nimg:
                load_quant1(n + 2)
        conv2(nimg - 2)
        final(nimg - 2)
        conv2(nimg - 1)
        final(nimg - 1)

    nc.compile()
    return nc


@lru_cache(maxsize=1)
def _get_nc():
    return build_nc(NIMG)


def kernel(x, w1, w2, gamma1, beta1, mean1, var1,
           gamma2, beta2, mean2, var2, _trace=False):
    f = lambda a: np.ascontiguousarray(np.asarray(a, dtype=np.float32))
    x = f(x)
    n_total = x.shape[0]
    assert n_total == N_CORES * NIMG, x.shape
    xs = x.reshape(N_CORES, NIMG, P, H, W)
    rep = {
        "w1": f(w1), "w2": f(w2),
        "gamma1": f(gamma1), "beta1": f(beta1), "mean1": f(mean1), "var1": f(var1),
        "gamma2": f(gamma2), "beta2": f(beta2), "mean2": f(mean2), "var2": f(var2),
    }
    in_maps = [{"x": np.ascontiguousarray(xs[c]), **rep} for c in range(N_CORES)]
    nc = _get_nc()
    res = run_bass_kernel_spmd(nc, in_maps, core_ids=list(range(N_CORES)),
                               trace=_trace)
    out = np.concatenate([res.results[c]["out"] for c in range(N_CORES)], axis=0)
    if _trace:
        kernel.last_result = res
    return out.reshape(n_total, P, H, W)
